# revision 3
# baseline (speedup 1.0000x reference)
"""Causal self-attention (B=8, T=1500, C=256, H=8, D=32) on 8 trn2 NeuronCores.

Sharding: data-parallel over batch B — core b computes batch element b
end-to-end (no collectives). The host only re-lays-out inputs (transposes /
replication); every FLOP of the module runs on device.

v2 changes vs baseline (148us):
  - exp split across ScalarE (native spline exp) and VectorE (Schraudolph
    bit-trick: psum + B -> int16 -> bitcast bf16), removing the single-engine
    exp wall (81.8us serialized on ScalarE).  The 1/sqrt(D)*log2e*128 factor
    is folded into Wq on the host so the DVE op is a single tensor_scalar.
  - S matmuls as 16 32x32 array tiles (4 heads x 4 k-substrips) for full
    array concurrency instead of 4 row-tiled 32x128 matmuls.
  - output projection computed transposed (out^T[c,t]) so the bias add is
    per-partition on ScalarE and the result DMAs as bf16 (host re-transposes).
  - yd has no memset: first PV matmul per region uses start=True.
  - all input DMAs as large contiguous transfers on both HWDGE rings
    (sync+scalar); gpsimd only does memsets; dense PE warmup for HAM ramp.
"""

import numpy as np

B, T, C = 8, 1500, 256
H, D = 8, 32
SCALE = 1.0 / float(np.sqrt(D))
LOG2E = 1.4426950408889634
ALPHA = SCALE * LOG2E * 128.0          # folded into Wq/bq host-side
EXP_SCALE = float(np.log(2.0) / 128.0)  # ScalarE exp scale on alpha-scores
SCHRAUD_B = 16251.0                     # 127*128 + c, c=-5 calibrated
FRAC_DVE = 0.6                          # fraction of s4b columns on DVE
S_TILE16 = True
N_CORES = 8

Q_TILES = [(0, 512), (512, 512), (1024, 476)]
K_TILES = [(j * 128, min(128, T - j * 128)) for j in range(12)]
T_TILES = K_TILES

_CACHE = {}


def _build():
    import concourse.bass as bass
    import concourse.mybir as mybir
    import concourse.tile as tile
    from concourse import bacc

    f32 = mybir.dt.float32
    bf16 = mybir.dt.bfloat16
    i16 = mybir.dt.int16
    AF = mybir.ActivationFunctionType
    ALU = mybir.AluOpType

    nc = bacc.Bacc()

    xt_d = [
        nc.dram_tensor(f"xt{i}", [128, 2, nn], bf16, kind="ExternalInput")
        for i, (n0, nn) in enumerate(Q_TILES)
    ]
    wq_d = nc.dram_tensor("wq", [128, 2, C], bf16, kind="ExternalInput")
    wk_d = nc.dram_tensor("wk", [128, 2, C], bf16, kind="ExternalInput")
    wv_d = nc.dram_tensor("wv", [128, 2, C], bf16, kind="ExternalInput")
    wpx_d = nc.dram_tensor("wpx", [128, 4, C], bf16, kind="ExternalInput")
    bia_d = nc.dram_tensor("bia", [128, 6], f32, kind="ExternalInput")
    bvm_d = nc.dram_tensor("bvm", [128, 512], bf16, kind="ExternalInput")
    out_d = nc.dram_tensor("outT", [128, 2, T], bf16, kind="ExternalOutput")

    from contextlib import ExitStack

    with tile.TileContext(nc) as tc, ExitStack() as stack:
        pp = stack.enter_context(tc.tile_pool(name="persist", bufs=1))
        xt = pp.tile([128, 2, T], bf16, name="xt")
        wq_s = pp.tile([128, 2, C], bf16, name="wq_s")
        wk_s = pp.tile([128, 2, C], bf16, name="wk_s")
        wv_s = pp.tile([128, 2, C], bf16, name="wv_s")
        wpx_s = pp.tile([128, 4, C], bf16, name="wpx_s")
        bia_s = pp.tile([128, 6], f32, name="bia_s")
        bv_s = pp.tile([128, C], bf16, name="bv_s")
        msk_s = pp.tile([128, 2, 128], bf16, name="msk_s")
        qt0 = pp.tile([128, T], bf16, name="qt0")
        qt1 = pp.tile([128, T], bf16, name="qt1")
        kt0 = pp.tile([128, T], bf16, name="kt0")
        kt1 = pp.tile([128, T], bf16, name="kt1")
        qt, kt = [qt0, qt1], [kt0, kt1]
        # v + ones columns: per k-block, per head, 64 cols =
        # [v_d0-15 | 1s x16 | v_d16-31 | 1s x16] so PV also yields denominators
        vnat = pp.tile([128, 12, 8, 2, 2, 16], bf16, name="vnat")
        ytx_s = pp.tile([128, 4, T], bf16, name="ytx_s")
        warm = pp.tile([128, 640], bf16, name="warm")

        # ---------------- memsets on gpsimd (frees DVE) ----------------
        nc.gpsimd.memset(warm[:, :], 0.125)
        for tt in range(12):
            nc.gpsimd.memset(vnat[:, tt, :, :, :, :], 1.0)

        # ---------------- input DMAs: big transfers, both HWDGE rings -----
        nc.sync.dma_start(out=wq_s, in_=wq_d[:, :, :])
        nc.scalar.dma_start(out=wk_s, in_=wk_d[:, :, :])
        nc.scalar.dma_start(out=bia_s, in_=bia_d[:, :])
        nc.sync.dma_start(out=xt[:, 0:1, 0:512], in_=xt_d[0][:, 0:1, :])
        nc.scalar.dma_start(out=xt[:, 1:2, 0:512], in_=xt_d[0][:, 1:2, :])
        nc.sync.dma_start(out=wv_s, in_=wv_d[:, :, :])
        nc.sync.dma_start(
            out=msk_s[:, :, :],
            in_=bvm_d[:, 256:512].rearrange("p (a b) -> p a b", a=2),
        )
        nc.sync.dma_start(out=bv_s, in_=bvm_d[:, 0:256])
        for i, (n0, nn) in list(enumerate(Q_TILES))[1:]:
            nc.sync.dma_start(out=xt[:, 0:1, n0 : n0 + nn], in_=xt_d[i][:, 0:1, :])
            nc.scalar.dma_start(out=xt[:, 1:2, n0 : n0 + nn], in_=xt_d[i][:, 1:2, :])
        nc.scalar.dma_start(out=wpx_s, in_=wpx_d[:, :, :])

        # warm the ACT exp table before the real exps need it
        nc.scalar.activation(warm[:, 636:640], warm[:, 632:636], AF.Exp)

        # ---------------- PSUM pools ----------------
        es = stack.enter_context(tc.tile_pool(name="es", bufs=1))
        rr = stack.enter_context(tc.tile_pool(name="rr", bufs=2))
        ot = stack.enter_context(tc.tile_pool(name="ot", bufs=3))
        pj_ctx = tc.tile_pool(name="pj", bufs=1, space="PSUM")
        pjp = [pj_ctx.__enter__()]

        # dense warmup matmuls: ramp HAM to 2.4GHz during the DMA window
        wmm = pjp[0].tile([128, 512], f32, name="wmm", tag="pj", bufs=8)
        for _ in range(24):
            nc.tensor.matmul(
                out=wmm[:, 0:512],
                lhsT=warm[:, 0:128],
                rhs=warm[:, 128:640],
                start=True,
                stop=True,
            )

        def _ptile():
            return pjp[0].tile([128, 512], f32, name="pt", tag="pj", bufs=8)

        def emit_proj(n, vts):
            n0, nn = Q_TILES[n]
            for m in range(2):
                qp = _ptile()
                for kk in range(2):
                    nc.tensor.matmul(
                        out=qp[:, 0:nn],
                        lhsT=wq_s[:, kk, m * 128 : (m + 1) * 128],
                        rhs=xt[:, kk, n0 : n0 + nn],
                        start=(kk == 0),
                        stop=(kk == 1),
                    )
                nc.scalar.add(qt[m][:, n0 : n0 + nn], qp[:, 0:nn], bia_s[:, m : m + 1])
                kp = _ptile()
                for kk in range(2):
                    nc.tensor.matmul(
                        out=kp[:, 0:nn],
                        lhsT=wk_s[:, kk, m * 128 : (m + 1) * 128],
                        rhs=xt[:, kk, n0 : n0 + nn],
                        start=(kk == 0),
                        stop=(kk == 1),
                    )
                nc.scalar.add(
                    kt[m][:, n0 : n0 + nn], kp[:, 0:nn], bia_s[:, 2 + m : 3 + m]
                )
                for tt in vts[m::2]:
                    t0, tl = T_TILES[tt]
                    vp = _ptile()
                    for kk in range(2):
                        nc.tensor.matmul(
                            out=vp[0:tl, 0:C],
                            lhsT=xt[:, kk, t0 : t0 + tl],
                            rhs=wv_s[:, kk, :],
                            start=(kk == 0),
                            stop=(kk == 1),
                        )
                    nc.vector.tensor_tensor(
                        out=vnat[0:tl, tt, :, :, 0, :],
                        in0=vp[0:tl, 0:C].rearrange(
                            "p (h half d) -> p h half d", h=8, half=2
                        ),
                        in1=bv_s[0:tl, :].rearrange(
                            "p (h half d) -> p h half d", h=8, half=2
                        ),
                        op=ALU.add,
                    )

        shuf = [16 + (i % 16) for i in range(32)]

        def emit_attn(qi, g):
            q0, qn = Q_TILES[qi]
            yd = ps.tile([128, 2, 512], f32, name="yd", tag="yd", bufs=1)

            js = [j for j, (k0, kn) in enumerate(K_TILES) if k0 <= q0 + qn - 1]
            jlast = js[-1]
            jfirst = js[0]

            def emit_S(j):
                k0, kn = K_TILES[j]
                r = max(0, k0 - q0)
                s4a = ps.tile([128, 2, 512], f32, name="s4a", tag="s4a", bufs=2)
                s4b = ps.tile([128, 2, 512], f32, name="s4b", tag="s4b", bufs=1)
                if S_TILE16:
                    for hh in range(4):
                        dst = s4a if hh < 2 else s4b
                        for ss in range(4):
                            ms = min(32, kn - 32 * ss)
                            if ms <= 0:
                                break
                            nc.tensor.matmul(
                                out=dst[32 * ss : 32 * ss + ms, hh % 2, r:qn],
                                lhsT=kt[g][
                                    32 * hh : 32 * (hh + 1),
                                    k0 + 32 * ss : k0 + 32 * ss + ms,
                                ],
                                rhs=qt[g][32 * hh : 32 * (hh + 1), q0 + r : q0 + qn],
                                start=True,
                                stop=True,
                                tile_position=(32 * hh, 32 * ss),
                            )
                else:
                    for hh in range(4):
                        dst = s4a if hh < 2 else s4b
                        nc.tensor.matmul(
                            out=dst[0:kn, hh % 2, r:qn],
                            lhsT=kt[g][32 * hh : 32 * (hh + 1), k0 : k0 + kn],
                            rhs=qt[g][32 * hh : 32 * (hh + 1), q0 + r : q0 + qn],
                            start=True,
                            stop=True,
                            tile_position=(32 * hh, 0),
                        )
                return s4a, s4b

            cur = emit_S(js[0])
            for idx, j in enumerate(js):
                k0, kn = K_TILES[j]
                r = max(0, k0 - q0)
                diag = k0 >= q0
                w = min(kn, qn - r) if diag else 0
                nxt = emit_S(js[idx + 1]) if idx + 1 < len(js) else None
                s4a, s4b = cur
                esl_a = es.tile([128, 2, 512], bf16, name="esl_a", tag="esl_a", bufs=3)
                esl_b = es.tile([128, 2, 512], bf16, name="esl_b", tag="esl_b", bufs=3)
                # column split of s4b between ScalarE (exact) and DVE (approx)
                mid = r + int(np.ceil((qn - r) * (1.0 - FRAC_DVE)))
                if diag:
                    mid = max(mid, r + w)
                mid = min(mid, qn)
                nc.scalar.activation(
                    out=esl_a[0:kn, :, r:qn], in_=s4a[0:kn, :, r:qn],
                    func=AF.Exp, scale=EXP_SCALE,
                )
                if mid > r:
                    nc.scalar.activation(
                        out=esl_b[0:kn, :, r:mid], in_=s4b[0:kn, :, r:mid],
                        func=AF.Exp, scale=EXP_SCALE,
                    )
                if mid < qn:
                    nc.vector.tensor_scalar(
                        out=esl_b[0:kn, :, mid:qn].bitcast(i16),
                        in0=s4b[0:kn, :, mid:qn],
                        scalar1=SCHRAUD_B,
                        scalar2=0.0,
                        op0=ALU.add,
                        op1=ALU.max,
                    )
                if diag:
                    for esl in (esl_a, esl_b):
                        nc.vector.tensor_tensor(
                            out=esl[0:kn, :, r : r + w],
                            in0=esl[0:kn, :, r : r + w],
                            in1=msk_s[0:kn, :, 0:w],
                            op=ALU.mult,
                        )
                for hh in range(4):
                    esl = esl_a if hh < 2 else esl_b
                    pr, hl = hh // 2, hh % 2
                    head = 4 * g + hh
                    nc.tensor.matmul(
                        out=yd[64 * hl : 64 * (hl + 1), pr, r:qn],
                        lhsT=vnat[0:kn, j, head, :, :, :],
                        rhs=esl[0:kn, hl, r:qn],
                        start=(j == jfirst),
                        stop=(j == jlast),
                        tile_position=(0, 64 * hl),
                        skip_group_check=True,
                    )
                cur = nxt
            # normalization: broadcast denominator lanes, approx-recip, mult
            rs = rr.tile([128, 2, 512], f32, name="rs", tag="rs", bufs=2)
            nc.vector.stream_shuffle(
                out=rs[:, :, 0:qn], in_=yd[:, :, 0:qn], mask=shuf
            )
            rq = rr.tile([128, 2, 512], f32, name="rq", tag="rq", bufs=2)
            nc.vector.reciprocal_approx_fast(out=rq[:, :, 0:qn], in_=rs[:, :, 0:qn])
            nc.vector.tensor_tensor(
                out=ytx_s[:, 2 * g : 2 * g + 2, q0 : q0 + qn],
                in0=yd[:, :, 0:qn],
                in1=rq[:, :, 0:qn],
                op=ALU.mult,
            )

        def emit_outproj(qi):
            q0, qn = Q_TILES[qi]
            for m in range(2):
                ops = ps.tile([128, 512], f32, name="ops", tag="s4a", bufs=2)
                for sl in range(4):
                    nc.tensor.matmul(
                        out=ops[:, 0:qn],
                        lhsT=wpx_s[:, sl, m * 128 : (m + 1) * 128],
                        rhs=ytx_s[:, sl, q0 : q0 + qn],
                        start=(sl == 0),
                        stop=(sl == 3),
                    )
                ost = ot.tile([128, 512], bf16, name="ost", tag="ost")
                nc.scalar.add(ost[:, 0:qn], ops[:, 0:qn], bia_s[:, 4 + m : 5 + m])
                nc.sync.dma_start(
                    out=out_d[:, m, q0 : q0 + qn], in_=ost[:, 0:qn]
                )

        emit_proj(0, list(range(0, 4)))
        emit_proj(1, list(range(4, 8)))
        emit_proj(2, list(range(8, 12)))
        pj_ctx.__exit__(None, None, None)
        ps = stack.enter_context(tc.tile_pool(name="ps", bufs=1, space="PSUM"))
        for qi in range(3):
            emit_attn(qi, 0)
            emit_attn(qi, 1)
            emit_outproj(qi)

    nc.compile()
    return nc


def _get_nc():
    if "nc" not in _CACHE:
        _CACHE["nc"] = _build()
    return _CACHE["nc"]


def _make_in_maps(inputs):
    f = np.float32
    x = np.asarray(inputs["x"], f)
    Wq = np.asarray(inputs["Wq"], f)
    Wk = np.asarray(inputs["Wk"], f)
    Wv = np.asarray(inputs["Wv"], f)
    Wp = np.asarray(inputs["Wp"], f)
    bq = np.asarray(inputs["bq"], f)
    bk = np.asarray(inputs["bk"], f)
    bv = np.asarray(inputs["bv"], f)
    bp = np.asarray(inputs["bp"], f)

    import ml_dtypes

    bf = ml_dtypes.bfloat16
    tri = np.triu(np.ones((128, 128), f))  # keep where k-row <= q-col

    # Wp^T rows permuted to the scattered y^T-slab layout (v/ones interleave)
    wpx = np.zeros((128, 4, C), f)
    for i in range(4):
        g, pr = divmod(i, 2)
        for p in range(128):
            hl, ppp = divmod(p, 64)
            head = 4 * g + 2 * pr + hl
            half, inner = divmod(ppp, 32)
            if inner < 16:
                d = half * 16 + inner
                wpx[p, i, :] = Wp[:, head * 32 + d]

    wqt = np.ascontiguousarray((Wq.T * ALPHA).reshape(2, 128, C).transpose(1, 0, 2))
    wkt = np.ascontiguousarray(Wk.T.reshape(2, 128, C).transpose(1, 0, 2))
    wvt = np.ascontiguousarray(Wv.T.reshape(2, 128, C).transpose(1, 0, 2))

    bia = np.zeros((128, 6), f)
    bia[:, 0] = bq[0:128] * ALPHA
    bia[:, 1] = bq[128:256] * ALPHA
    bia[:, 2] = bk[0:128]
    bia[:, 3] = bk[128:256]
    bia[:, 4] = bp[0:128]
    bia[:, 5] = bp[128:256]

    bvm = np.zeros((128, 512), f)
    bvm[:, 0:256] = np.tile(bv, (128, 1))
    bvm[:, 256:384] = tri
    bvm[:, 384:512] = tri

    common = {
        "wq": wqt.astype(bf),
        "wk": wkt.astype(bf),
        "wv": wvt.astype(bf),
        "wpx": np.ascontiguousarray(wpx).astype(bf),
        "bia": np.ascontiguousarray(bia),
        "bvm": np.ascontiguousarray(bvm).astype(bf),
    }
    maps = []
    for b in range(N_CORES):
        xtb = x[b].T.reshape(2, 128, T).transpose(1, 0, 2)  # [128, 2, T]
        m = dict(common)
        for i, (n0, nn) in enumerate(Q_TILES):
            m[f"xt{i}"] = np.ascontiguousarray(xtb[:, :, n0 : n0 + nn]).astype(bf)
        maps.append(m)
    return maps


def run(inputs, trace=False):
    from concourse.bass_utils import run_bass_kernel_spmd

    nc = _get_nc()
    in_maps = _make_in_maps(inputs)
    res = run_bass_kernel_spmd(nc, in_maps, list(range(N_CORES)), trace=trace)
    outs = []
    for i in range(N_CORES):
        ot = np.asarray(res.results[i]["outT"], dtype=np.float32)  # [128, 2, T]
        outs.append(ot.transpose(2, 1, 0).reshape(T, C))
    return np.stack(outs, axis=0), res


def kernel(**inputs) -> np.ndarray:
    out, _ = run(inputs, trace=False)
    return out


# revision 8
# speedup vs baseline: 1.0266x; 1.0266x over previous
"""Causal self-attention (B=8, T=1500, C=256, H=8, D=32) on 8 trn2 NeuronCores.

Sharding: data-parallel over batch B — core b computes batch element b
end-to-end (no collectives). The host only re-lays-out inputs (transposes /
replication); every FLOP of the module runs on device.

v2 changes vs baseline (148us):
  - exp split across ScalarE (native spline exp) and VectorE (Schraudolph
    bit-trick: psum + B -> int16 -> bitcast bf16), removing the single-engine
    exp wall (81.8us serialized on ScalarE).  The 1/sqrt(D)*log2e*128 factor
    is folded into Wq on the host so the DVE op is a single tensor_scalar.
  - S matmuls as 16 32x32 array tiles (4 heads x 4 k-substrips) for full
    array concurrency instead of 4 row-tiled 32x128 matmuls.
  - output projection computed transposed (out^T[c,t]) so the bias add is
    per-partition on ScalarE and the result DMAs as bf16 (host re-transposes).
  - yd has no memset: first PV matmul per region uses start=True.
  - all input DMAs as large contiguous transfers on both HWDGE rings
    (sync+scalar); gpsimd only does memsets; dense PE warmup for HAM ramp.
"""

import numpy as np

B, T, C = 8, 1500, 256
H, D = 8, 32
SCALE = 1.0 / float(np.sqrt(D))
LOG2E = 1.4426950408889634
ALPHA = SCALE * LOG2E * 128.0          # folded into Wq/bq host-side
EXP_SCALE = float(np.log(2.0) / 128.0)  # ScalarE exp scale on alpha-scores
SCHRAUD_B = 16251.0                     # 127*128 + c, c=-5 calibrated
FRAC_DVE = 0.6                          # fraction of s4b columns on DVE
S_TILE16 = True
N_CORES = 8

Q_TILES = [(0, 512), (512, 512), (1024, 476)]
K_TILES = [(j * 128, min(128, T - j * 128)) for j in range(12)]
T_TILES = K_TILES

_CACHE = {}


def _build():
    import concourse.bass as bass
    import concourse.mybir as mybir
    import concourse.tile as tile
    from concourse import bacc

    f32 = mybir.dt.float32
    bf16 = mybir.dt.bfloat16
    i16 = mybir.dt.int16
    AF = mybir.ActivationFunctionType
    ALU = mybir.AluOpType

    nc = bacc.Bacc()

    xt_d = [
        nc.dram_tensor(f"xt{i}", [128, 2, nn], bf16, kind="ExternalInput")
        for i, (n0, nn) in enumerate(Q_TILES)
    ]
    wq_d = nc.dram_tensor("wq", [128, 2, C], bf16, kind="ExternalInput")
    wk_d = nc.dram_tensor("wk", [128, 2, C], bf16, kind="ExternalInput")
    wv_d = nc.dram_tensor("wv", [128, 2, C], bf16, kind="ExternalInput")
    wpx_d = nc.dram_tensor("wpx", [128, 4, C], bf16, kind="ExternalInput")
    bia_d = nc.dram_tensor("bia", [128, 6], f32, kind="ExternalInput")
    bvm_d = nc.dram_tensor("bvm", [128, 512], bf16, kind="ExternalInput")
    out_d = nc.dram_tensor("outT", [128, 2, T], bf16, kind="ExternalOutput")

    from contextlib import ExitStack

    with tile.TileContext(nc) as tc, ExitStack() as stack:
        pp = stack.enter_context(tc.tile_pool(name="persist", bufs=1))
        xt = pp.tile([128, 2, T], bf16, name="xt")
        wq_s = pp.tile([128, 2, C], bf16, name="wq_s")
        wk_s = pp.tile([128, 2, C], bf16, name="wk_s")
        wv_s = pp.tile([128, 2, C], bf16, name="wv_s")
        wpx_s = pp.tile([128, 4, C], bf16, name="wpx_s")
        bia_s = pp.tile([128, 6], f32, name="bia_s")
        bv_s = pp.tile([128, C], bf16, name="bv_s")
        msk_s = pp.tile([128, 2, 128], bf16, name="msk_s")
        qt0 = pp.tile([128, T], bf16, name="qt0")
        qt1 = pp.tile([128, T], bf16, name="qt1")
        kt0 = pp.tile([128, T], bf16, name="kt0")
        kt1 = pp.tile([128, T], bf16, name="kt1")
        qt, kt = [qt0, qt1], [kt0, kt1]
        # v + ones columns: per k-block, per head, 64 cols =
        # [v_d0-15 | 1s x16 | v_d16-31 | 1s x16] so PV also yields denominators
        vnat = pp.tile([128, 12, 8, 2, 2, 16], bf16, name="vnat")
        ytx_s = pp.tile([128, 4, T], bf16, name="ytx_s")
        warm = pp.tile([128, 640], bf16, name="warm")
        warm2 = pp.tile([128, 8], bf16, name="warm2")

        # ---------------- memsets on gpsimd (frees DVE) ----------------
        nc.gpsimd.memset(warm2[:, :], 0.125)
        nc.gpsimd.memset(warm[:, :], 0.125)
        for tt in range(12):
            nc.gpsimd.memset(vnat[:, tt, :, :, :, :], 1.0)

        # ---------------- input DMAs: big transfers, both HWDGE rings -----
        nc.sync.dma_start(out=wq_s, in_=wq_d[:, :, :])
        nc.scalar.dma_start(out=wk_s, in_=wk_d[:, :, :])
        nc.scalar.dma_start(out=bia_s, in_=bia_d[:, :])
        nc.sync.dma_start(out=xt[:, 0:1, 0:512], in_=xt_d[0][:, 0:1, :])
        nc.scalar.dma_start(out=xt[:, 1:2, 0:512], in_=xt_d[0][:, 1:2, :])
        nc.sync.dma_start(out=wv_s, in_=wv_d[:, :, :])
        nc.sync.dma_start(
            out=msk_s[:, :, :],
            in_=bvm_d[:, 256:512].rearrange("p (a b) -> p a b", a=2),
        )
        nc.sync.dma_start(out=bv_s, in_=bvm_d[:, 0:256])
        for i, (n0, nn) in list(enumerate(Q_TILES))[1:]:
            nc.sync.dma_start(out=xt[:, 0:1, n0 : n0 + nn], in_=xt_d[i][:, 0:1, :])
            nc.scalar.dma_start(out=xt[:, 1:2, n0 : n0 + nn], in_=xt_d[i][:, 1:2, :])
        nc.scalar.dma_start(out=wpx_s, in_=wpx_d[:, :, :])

        # warm the ACT exp table before the real exps need it
        nc.scalar.activation(warm2[:, 4:8], warm2[:, 0:4], AF.Exp)

        # ---------------- PSUM pools ----------------
        es = stack.enter_context(tc.tile_pool(name="es", bufs=1))
        rr = stack.enter_context(tc.tile_pool(name="rr", bufs=2))
        ot = stack.enter_context(tc.tile_pool(name="ot", bufs=3))
        pj_ctx = tc.tile_pool(name="pj", bufs=1, space="PSUM")
        pjp = [pj_ctx.__enter__()]

        # dense warmup matmuls: ramp HAM to 2.4GHz during the DMA window
        wmm = pjp[0].tile([128, 512], f32, name="wmm", tag="pj", bufs=8)
        for _ in range(8):
            nc.tensor.matmul(
                out=wmm[:, 0:512],
                lhsT=warm[:, 0:128],
                rhs=warm[:, 128:640],
                start=True,
                stop=True,
            )

        def _ptile():
            return pjp[0].tile([128, 512], f32, name="pt", tag="pj", bufs=8)

        def emit_proj(n, vts):
            n0, nn = Q_TILES[n]
            for m in range(2):
                qp = _ptile()
                for kk in range(2):
                    nc.tensor.matmul(
                        out=qp[:, 0:nn],
                        lhsT=wq_s[:, kk, m * 128 : (m + 1) * 128],
                        rhs=xt[:, kk, n0 : n0 + nn],
                        start=(kk == 0),
                        stop=(kk == 1),
                    )
                nc.scalar.add(qt[m][:, n0 : n0 + nn], qp[:, 0:nn], bia_s[:, m : m + 1])
                kp = _ptile()
                for kk in range(2):
                    nc.tensor.matmul(
                        out=kp[:, 0:nn],
                        lhsT=wk_s[:, kk, m * 128 : (m + 1) * 128],
                        rhs=xt[:, kk, n0 : n0 + nn],
                        start=(kk == 0),
                        stop=(kk == 1),
                    )
                nc.scalar.add(
                    kt[m][:, n0 : n0 + nn], kp[:, 0:nn], bia_s[:, 2 + m : 3 + m]
                )
                for tt in vts[m::2]:
                    t0, tl = T_TILES[tt]
                    vp = _ptile()
                    for kk in range(2):
                        nc.tensor.matmul(
                            out=vp[0:tl, 0:C],
                            lhsT=xt[:, kk, t0 : t0 + tl],
                            rhs=wv_s[:, kk, :],
                            start=(kk == 0),
                            stop=(kk == 1),
                        )
                    nc.vector.tensor_tensor(
                        out=vnat[0:tl, tt, :, :, 0, :],
                        in0=vp[0:tl, 0:C].rearrange(
                            "p (h half d) -> p h half d", h=8, half=2
                        ),
                        in1=bv_s[0:tl, :].rearrange(
                            "p (h half d) -> p h half d", h=8, half=2
                        ),
                        op=ALU.add,
                    )

        shuf = [16 + (i % 16) for i in range(32)]

        def emit_attn(qi, g):
            q0, qn = Q_TILES[qi]
            yd = ps.tile([128, 2, 512], f32, name="yd", tag="yd", bufs=1)

            js = [j for j, (k0, kn) in enumerate(K_TILES) if k0 <= q0 + qn - 1]
            jlast = js[-1]
            jfirst = js[0]

            def emit_S(j):
                k0, kn = K_TILES[j]
                r = max(0, k0 - q0)
                s4a = ps.tile([128, 2, 512], f32, name="s4a", tag="s4a", bufs=2)
                s4b = ps.tile([128, 2, 512], f32, name="s4b", tag="s4b", bufs=1)
                if S_TILE16:
                    for hh in range(4):
                        dst = s4a if hh < 2 else s4b
                        for ss in range(4):
                            ms = min(32, kn - 32 * ss)
                            if ms <= 0:
                                break
                            nc.tensor.matmul(
                                out=dst[32 * ss : 32 * ss + ms, hh % 2, r:qn],
                                lhsT=kt[g][
                                    32 * hh : 32 * (hh + 1),
                                    k0 + 32 * ss : k0 + 32 * ss + ms,
                                ],
                                rhs=qt[g][32 * hh : 32 * (hh + 1), q0 + r : q0 + qn],
                                start=True,
                                stop=True,
                                tile_position=(32 * hh, 32 * ss),
                            )
                else:
                    for hh in range(4):
                        dst = s4a if hh < 2 else s4b
                        nc.tensor.matmul(
                            out=dst[0:kn, hh % 2, r:qn],
                            lhsT=kt[g][32 * hh : 32 * (hh + 1), k0 : k0 + kn],
                            rhs=qt[g][32 * hh : 32 * (hh + 1), q0 + r : q0 + qn],
                            start=True,
                            stop=True,
                            tile_position=(32 * hh, 0),
                        )
                return s4a, s4b

            cur = emit_S(js[0])
            for idx, j in enumerate(js):
                k0, kn = K_TILES[j]
                r = max(0, k0 - q0)
                diag = k0 >= q0
                w = min(kn, qn - r) if diag else 0
                nxt = emit_S(js[idx + 1]) if idx + 1 < len(js) else None
                s4a, s4b = cur
                esl_a = es.tile([128, 2, 512], bf16, name="esl_a", tag="esl_a", bufs=3)
                esl_b = es.tile([128, 2, 512], bf16, name="esl_b", tag="esl_b", bufs=3)
                # column split of s4b between ScalarE (exact) and DVE (approx)
                mid = r + int(np.ceil((qn - r) * (1.0 - FRAC_DVE)))
                if diag:
                    mid = max(mid, r + w)
                mid = min(mid, qn)
                nc.scalar.activation(
                    out=esl_a[0:kn, :, r:qn], in_=s4a[0:kn, :, r:qn],
                    func=AF.Exp, scale=EXP_SCALE,
                )
                if mid > r:
                    nc.scalar.activation(
                        out=esl_b[0:kn, :, r:mid], in_=s4b[0:kn, :, r:mid],
                        func=AF.Exp, scale=EXP_SCALE,
                    )
                if mid < qn:
                    nc.vector.tensor_scalar(
                        out=esl_b[0:kn, :, mid:qn].bitcast(i16),
                        in0=s4b[0:kn, :, mid:qn],
                        scalar1=SCHRAUD_B,
                        scalar2=0.0,
                        op0=ALU.add,
                        op1=ALU.max,
                    )
                if diag:
                    # 0/1 mask strip on gpsimd (SBUF-only) — keeps the DVE
                    # queue clear at the group tail where norm must run
                    for esl in (esl_a, esl_b):
                        nc.gpsimd.tensor_tensor(
                            out=esl[0:kn, :, r : r + w],
                            in0=esl[0:kn, :, r : r + w],
                            in1=msk_s[0:kn, :, 0:w],
                            op=ALU.mult,
                        )
                for hh in range(4):
                    esl = esl_a if hh < 2 else esl_b
                    pr, hl = hh // 2, hh % 2
                    head = 4 * g + hh
                    nc.tensor.matmul(
                        out=yd[64 * hl : 64 * (hl + 1), pr, r:qn],
                        lhsT=vnat[0:kn, j, head, :, :, :],
                        rhs=esl[0:kn, hl, r:qn],
                        start=(j == jfirst),
                        stop=(j == jlast),
                        tile_position=(0, 64 * hl),
                        skip_group_check=True,
                    )
                cur = nxt
            # normalization: broadcast denominator lanes, approx-recip, mult
            rs = rr.tile([128, 2, 512], f32, name="rs", tag="rs", bufs=2)
            nc.vector.stream_shuffle(
                out=rs[:, :, 0:qn], in_=yd[:, :, 0:qn], mask=shuf
            )
            rq = rr.tile([128, 2, 512], f32, name="rq", tag="rq", bufs=2)
            nc.vector.reciprocal_approx_fast(out=rq[:, :, 0:qn], in_=rs[:, :, 0:qn])
            nc.vector.tensor_tensor(
                out=ytx_s[:, 2 * g : 2 * g + 2, q0 : q0 + qn],
                in0=yd[:, :, 0:qn],
                in1=rq[:, :, 0:qn],
                op=ALU.mult,
            )

        def emit_outproj(qi):
            # column-split so evac/DMA pipeline; DMAs split over both rings
            q0, qn = Q_TILES[qi]
            half = (qn + 1) // 2
            for m in range(2):
                ops = ps.tile([128, 512], f32, name="ops", tag="s4a", bufs=2)
                for ci, (c0, cn) in enumerate([(0, half), (half, qn - half)]):
                    for sl in range(4):
                        nc.tensor.matmul(
                            out=ops[:, c0 : c0 + cn],
                            lhsT=wpx_s[:, sl, m * 128 : (m + 1) * 128],
                            rhs=ytx_s[:, sl, q0 + c0 : q0 + c0 + cn],
                            start=(sl == 0),
                            stop=(sl == 3),
                        )
                    ost = ot.tile([128, 256], bf16, name="ost", tag="ost")
                    nc.scalar.add(
                        ost[:, 0:cn], ops[:, c0 : c0 + cn], bia_s[:, 4 + m : 5 + m]
                    )
                    eng = nc.sync if (m + ci) % 2 == 0 else nc.scalar
                    eng.dma_start(
                        out=out_d[:, m, q0 + c0 : q0 + c0 + cn], in_=ost[:, 0:cn]
                    )

        emit_proj(0, list(range(0, 4)))
        emit_proj(1, list(range(4, 8)))
        emit_proj(2, list(range(8, 12)))
        pj_ctx.__exit__(None, None, None)
        ps = stack.enter_context(tc.tile_pool(name="ps", bufs=1, space="PSUM"))
        emit_attn(0, 0)
        emit_attn(0, 1)
        emit_attn(1, 0)
        emit_outproj(0)
        emit_attn(1, 1)
        emit_attn(2, 0)
        emit_outproj(1)
        emit_attn(2, 1)
        emit_outproj(2)

    nc.compile()
    return nc


def _get_nc():
    if "nc" not in _CACHE:
        _CACHE["nc"] = _build()
    return _CACHE["nc"]


def _make_in_maps(inputs):
    f = np.float32
    x = np.asarray(inputs["x"], f)
    Wq = np.asarray(inputs["Wq"], f)
    Wk = np.asarray(inputs["Wk"], f)
    Wv = np.asarray(inputs["Wv"], f)
    Wp = np.asarray(inputs["Wp"], f)
    bq = np.asarray(inputs["bq"], f)
    bk = np.asarray(inputs["bk"], f)
    bv = np.asarray(inputs["bv"], f)
    bp = np.asarray(inputs["bp"], f)

    import ml_dtypes

    bf = ml_dtypes.bfloat16
    tri = np.triu(np.ones((128, 128), f))  # keep where k-row <= q-col

    # Wp^T rows permuted to the scattered y^T-slab layout (v/ones interleave)
    wpx = np.zeros((128, 4, C), f)
    for i in range(4):
        g, pr = divmod(i, 2)
        for p in range(128):
            hl, ppp = divmod(p, 64)
            head = 4 * g + 2 * pr + hl
            half, inner = divmod(ppp, 32)
            if inner < 16:
                d = half * 16 + inner
                wpx[p, i, :] = Wp[:, head * 32 + d]

    wqt = np.ascontiguousarray((Wq.T * ALPHA).reshape(2, 128, C).transpose(1, 0, 2))
    wkt = np.ascontiguousarray(Wk.T.reshape(2, 128, C).transpose(1, 0, 2))
    wvt = np.ascontiguousarray(Wv.T.reshape(2, 128, C).transpose(1, 0, 2))

    bia = np.zeros((128, 6), f)
    bia[:, 0] = bq[0:128] * ALPHA
    bia[:, 1] = bq[128:256] * ALPHA
    bia[:, 2] = bk[0:128]
    bia[:, 3] = bk[128:256]
    bia[:, 4] = bp[0:128]
    bia[:, 5] = bp[128:256]

    bvm = np.zeros((128, 512), f)
    bvm[:, 0:256] = np.tile(bv, (128, 1))
    bvm[:, 256:384] = tri
    bvm[:, 384:512] = tri

    common = {
        "wq": wqt.astype(bf),
        "wk": wkt.astype(bf),
        "wv": wvt.astype(bf),
        "wpx": np.ascontiguousarray(wpx).astype(bf),
        "bia": np.ascontiguousarray(bia),
        "bvm": np.ascontiguousarray(bvm).astype(bf),
    }
    maps = []
    for b in range(N_CORES):
        xtb = x[b].T.reshape(2, 128, T).transpose(1, 0, 2)  # [128, 2, T]
        m = dict(common)
        for i, (n0, nn) in enumerate(Q_TILES):
            m[f"xt{i}"] = np.ascontiguousarray(xtb[:, :, n0 : n0 + nn]).astype(bf)
        maps.append(m)
    return maps


def run(inputs, trace=False):
    from concourse.bass_utils import run_bass_kernel_spmd

    nc = _get_nc()
    in_maps = _make_in_maps(inputs)
    res = run_bass_kernel_spmd(nc, in_maps, list(range(N_CORES)), trace=trace)
    outs = []
    for i in range(N_CORES):
        ot = np.asarray(res.results[i]["outT"], dtype=np.float32)  # [128, 2, T]
        outs.append(ot.transpose(2, 1, 0).reshape(T, C))
    return np.stack(outs, axis=0), res


def kernel(**inputs) -> np.ndarray:
    out, _ = run(inputs, trace=False)
    return out


# revision 11
# speedup vs baseline: 1.1266x; 1.0975x over previous
"""Causal self-attention (B=8, T=1500, C=256, H=8, D=32) on 8 trn2 NeuronCores.

Sharding: data-parallel over batch B — core b computes batch element b
end-to-end (no collectives). The host only re-lays-out inputs (transposes /
replication); every FLOP of the module runs on device.

v2 changes vs baseline (148us):
  - exp split across ScalarE (native spline exp) and VectorE (Schraudolph
    bit-trick: psum + B -> int16 -> bitcast bf16), removing the single-engine
    exp wall (81.8us serialized on ScalarE).  The 1/sqrt(D)*log2e*128 factor
    is folded into Wq on the host so the DVE op is a single tensor_scalar.
  - S matmuls as 16 32x32 array tiles (4 heads x 4 k-substrips) for full
    array concurrency instead of 4 row-tiled 32x128 matmuls.
  - output projection computed transposed (out^T[c,t]) so the bias add is
    per-partition on ScalarE and the result DMAs as bf16 (host re-transposes).
  - yd has no memset: first PV matmul per region uses start=True.
  - all input DMAs as large contiguous transfers on both HWDGE rings
    (sync+scalar); gpsimd only does memsets; dense PE warmup for HAM ramp.
"""

import numpy as np

B, T, C = 8, 1500, 256
H, D = 8, 32
SCALE = 1.0 / float(np.sqrt(D))
LOG2E = 1.4426950408889634
ALPHA = SCALE * LOG2E * 128.0          # folded into Wq/bq host-side
EXP_SCALE = float(np.log(2.0) / 128.0)  # ScalarE exp scale on alpha-scores
SCHRAUD_B = 16251.0                     # 127*128 + c, c=-5 calibrated
FRAC_DVE = 0.68                         # fraction of s4b columns on DVE
S_TILE16 = True
N_CORES = 8

Q_TILES = [(0, 512), (512, 512), (1024, 476)]
K_TILES = [(j * 128, min(128, T - j * 128)) for j in range(12)]
T_TILES = K_TILES

_CACHE = {}


def _build():
    import concourse.bass as bass
    import concourse.mybir as mybir
    import concourse.tile as tile
    from concourse import bacc

    f32 = mybir.dt.float32
    bf16 = mybir.dt.bfloat16
    i16 = mybir.dt.int16
    AF = mybir.ActivationFunctionType
    ALU = mybir.AluOpType

    nc = bacc.Bacc()

    xt_d = [
        nc.dram_tensor(f"xt{i}", [128, 2, nn], bf16, kind="ExternalInput")
        for i, (n0, nn) in enumerate(Q_TILES)
    ]
    wq_d = nc.dram_tensor("wq", [128, 2, C], bf16, kind="ExternalInput")
    wk_d = nc.dram_tensor("wk", [128, 2, C], bf16, kind="ExternalInput")
    wv_d = nc.dram_tensor("wv", [128, 2, C], bf16, kind="ExternalInput")
    wpx_d = nc.dram_tensor("wpx", [128, 4, C], bf16, kind="ExternalInput")
    bia_d = nc.dram_tensor("bia", [128, 6], f32, kind="ExternalInput")
    bvm_d = nc.dram_tensor("bvm", [128, 512], bf16, kind="ExternalInput")
    out_d = nc.dram_tensor("outT", [128, 2, T], bf16, kind="ExternalOutput")

    from contextlib import ExitStack

    with tile.TileContext(nc) as tc, ExitStack() as stack:
        pp = stack.enter_context(tc.tile_pool(name="persist", bufs=1))
        xt = pp.tile([128, 2, T], bf16, name="xt")
        wq_s = pp.tile([128, 2, C], bf16, name="wq_s")
        wk_s = pp.tile([128, 2, C], bf16, name="wk_s")
        wv_s = pp.tile([128, 2, C], bf16, name="wv_s")
        wpx_s = pp.tile([128, 4, C], bf16, name="wpx_s")
        bia_s = pp.tile([128, 6], f32, name="bia_s")
        bv_s = pp.tile([128, C], bf16, name="bv_s")
        msk_s = pp.tile([128, 2, 128], bf16, name="msk_s")
        qt0 = pp.tile([128, T], bf16, name="qt0")
        qt1 = pp.tile([128, T], bf16, name="qt1")
        kt0 = pp.tile([128, T], bf16, name="kt0")
        kt1 = pp.tile([128, T], bf16, name="kt1")
        qt, kt = [qt0, qt1], [kt0, kt1]
        # v + ones columns: per k-block, per head, 64 cols =
        # [v_d0-15 | 1s x16 | v_d16-31 | 1s x16] so PV also yields denominators
        vnat = pp.tile([128, 12, 8, 2, 2, 16], bf16, name="vnat")
        ytx_s = pp.tile([128, 4, T], bf16, name="ytx_s")
        warm = pp.tile([128, 640], bf16, name="warm")
        warm2 = pp.tile([128, 8], bf16, name="warm2")

        # ---------------- memsets on gpsimd (frees DVE) ----------------
        nc.gpsimd.memset(warm2[:, :], 0.125)
        nc.gpsimd.memset(warm[:, :], 0.125)
        for tt in range(12):
            nc.gpsimd.memset(vnat[:, tt, :, :, :, :], 1.0)

        # ---------------- input DMAs: big transfers, both HWDGE rings -----
        nc.sync.dma_start(out=wq_s, in_=wq_d[:, :, :])
        nc.scalar.dma_start(out=wk_s, in_=wk_d[:, :, :])
        nc.scalar.dma_start(out=bia_s, in_=bia_d[:, :])
        nc.sync.dma_start(out=xt[:, 0:1, 0:512], in_=xt_d[0][:, 0:1, :])
        nc.scalar.dma_start(out=xt[:, 1:2, 0:512], in_=xt_d[0][:, 1:2, :])
        nc.sync.dma_start(out=wv_s, in_=wv_d[:, :, :])
        nc.sync.dma_start(
            out=msk_s[:, :, :],
            in_=bvm_d[:, 256:512].rearrange("p (a b) -> p a b", a=2),
        )
        nc.sync.dma_start(out=bv_s, in_=bvm_d[:, 0:256])
        for i, (n0, nn) in list(enumerate(Q_TILES))[1:]:
            nc.sync.dma_start(out=xt[:, 0:1, n0 : n0 + nn], in_=xt_d[i][:, 0:1, :])
            nc.scalar.dma_start(out=xt[:, 1:2, n0 : n0 + nn], in_=xt_d[i][:, 1:2, :])
        nc.scalar.dma_start(out=wpx_s, in_=wpx_d[:, :, :])

        # warm the ACT exp table before the real exps need it
        nc.scalar.activation(warm2[:, 4:8], warm2[:, 0:4], AF.Exp)

        # ---------------- PSUM pools ----------------
        es = stack.enter_context(tc.tile_pool(name="es", bufs=1))
        rr = stack.enter_context(tc.tile_pool(name="rr", bufs=2))
        ot = stack.enter_context(tc.tile_pool(name="ot", bufs=3))
        pj_ctx = tc.tile_pool(name="pj", bufs=1, space="PSUM")
        pjp = [pj_ctx.__enter__()]

        # dense warmup matmuls: ramp HAM to 2.4GHz during the DMA window
        wmm = pjp[0].tile([128, 512], f32, name="wmm", tag="pj", bufs=8)
        for _ in range(8):
            nc.tensor.matmul(
                out=wmm[:, 0:512],
                lhsT=warm[:, 0:128],
                rhs=warm[:, 128:640],
                start=True,
                stop=True,
            )

        def _ptile():
            return pjp[0].tile([128, 512], f32, name="pt", tag="pj", bufs=8)

        def emit_proj(n, vts):
            n0, nn = Q_TILES[n]
            for m in range(2):
                qp = _ptile()
                for kk in range(2):
                    nc.tensor.matmul(
                        out=qp[:, 0:nn],
                        lhsT=wq_s[:, kk, m * 128 : (m + 1) * 128],
                        rhs=xt[:, kk, n0 : n0 + nn],
                        start=(kk == 0),
                        stop=(kk == 1),
                    )
                nc.scalar.add(qt[m][:, n0 : n0 + nn], qp[:, 0:nn], bia_s[:, m : m + 1])
                kp = _ptile()
                for kk in range(2):
                    nc.tensor.matmul(
                        out=kp[:, 0:nn],
                        lhsT=wk_s[:, kk, m * 128 : (m + 1) * 128],
                        rhs=xt[:, kk, n0 : n0 + nn],
                        start=(kk == 0),
                        stop=(kk == 1),
                    )
                nc.scalar.add(
                    kt[m][:, n0 : n0 + nn], kp[:, 0:nn], bia_s[:, 2 + m : 3 + m]
                )
                for tt in vts[m::2]:
                    t0, tl = T_TILES[tt]
                    vp = _ptile()
                    for kk in range(2):
                        nc.tensor.matmul(
                            out=vp[0:tl, 0:C],
                            lhsT=xt[:, kk, t0 : t0 + tl],
                            rhs=wv_s[:, kk, :],
                            start=(kk == 0),
                            stop=(kk == 1),
                        )
                    nc.vector.tensor_tensor(
                        out=vnat[0:tl, tt, :, :, 0, :],
                        in0=vp[0:tl, 0:C].rearrange(
                            "p (h half d) -> p h half d", h=8, half=2
                        ),
                        in1=bv_s[0:tl, :].rearrange(
                            "p (h half d) -> p h half d", h=8, half=2
                        ),
                        op=ALU.add,
                    )

        shuf = [16 + (i % 16) for i in range(32)]

        def emit_attn(qi, g):
            q0, qn = Q_TILES[qi]
            yd = ps.tile([128, 2, 512], f32, name="yd", tag="yd", bufs=1)

            js = [j for j, (k0, kn) in enumerate(K_TILES) if k0 <= q0 + qn - 1]
            jlast = js[-1]
            jfirst = js[0]

            def emit_S(j):
                k0, kn = K_TILES[j]
                r = max(0, k0 - q0)
                s4a = ps.tile([128, 2, 512], f32, name="s4a", tag="s4", bufs=3)
                s4b = ps.tile([128, 2, 512], f32, name="s4b", tag="s4", bufs=3)
                if S_TILE16:
                    # ss-outer so consecutive LDWEIGHTS hit different row
                    # groups (overlap with in-flight matmuls); 16 concurrent
                    # 32x32 array tiles
                    for ss in range(4):
                        ms = min(32, kn - 32 * ss)
                        if ms <= 0:
                            break
                        for hh in range(4):
                            dst = s4a if hh < 2 else s4b
                            nc.tensor.matmul(
                                out=dst[32 * ss : 32 * ss + ms, hh % 2, r:qn],
                                lhsT=kt[g][
                                    32 * hh : 32 * (hh + 1),
                                    k0 + 32 * ss : k0 + 32 * ss + ms,
                                ],
                                rhs=qt[g][32 * hh : 32 * (hh + 1), q0 + r : q0 + qn],
                                start=True,
                                stop=True,
                                tile_position=(32 * hh, 32 * ss),
                            )
                else:
                    for hh in range(4):
                        dst = s4a if hh < 2 else s4b
                        nc.tensor.matmul(
                            out=dst[0:kn, hh % 2, r:qn],
                            lhsT=kt[g][32 * hh : 32 * (hh + 1), k0 : k0 + kn],
                            rhs=qt[g][32 * hh : 32 * (hh + 1), q0 + r : q0 + qn],
                            start=True,
                            stop=True,
                            tile_position=(32 * hh, 0),
                        )
                return s4a, s4b

            cur = emit_S(js[0])
            for idx, j in enumerate(js):
                k0, kn = K_TILES[j]
                r = max(0, k0 - q0)
                diag = k0 >= q0
                w = min(kn, qn - r) if diag else 0
                nxt = emit_S(js[idx + 1]) if idx + 1 < len(js) else None
                s4a, s4b = cur
                esl_a = es.tile([128, 2, 512], bf16, name="esl_a", tag="esl_a", bufs=3)
                esl_b = es.tile([128, 2, 512], bf16, name="esl_b", tag="esl_b", bufs=3)
                # column split of s4b between ScalarE (exact) and DVE (approx)
                mid = r + int(np.ceil((qn - r) * (1.0 - FRAC_DVE)))
                if diag:
                    mid = max(mid, r + w)
                mid = min(mid, qn)
                nc.scalar.activation(
                    out=esl_a[0:kn, :, r:qn], in_=s4a[0:kn, :, r:qn],
                    func=AF.Exp, scale=EXP_SCALE,
                )
                if mid > r:
                    nc.scalar.activation(
                        out=esl_b[0:kn, :, r:mid], in_=s4b[0:kn, :, r:mid],
                        func=AF.Exp, scale=EXP_SCALE,
                    )
                if mid < qn:
                    nc.vector.tensor_scalar(
                        out=esl_b[0:kn, :, mid:qn].bitcast(i16),
                        in0=s4b[0:kn, :, mid:qn],
                        scalar1=SCHRAUD_B,
                        scalar2=0.0,
                        op0=ALU.add,
                        op1=ALU.max,
                    )
                if diag:
                    # 0/1 mask strip on gpsimd (SBUF-only) — keeps the DVE
                    # queue clear at the group tail where norm must run
                    for esl in (esl_a, esl_b):
                        nc.gpsimd.tensor_tensor(
                            out=esl[0:kn, :, r : r + w],
                            in0=esl[0:kn, :, r : r + w],
                            in1=msk_s[0:kn, :, 0:w],
                            op=ALU.mult,
                        )
                for hh in range(4):
                    esl = esl_a if hh < 2 else esl_b
                    pr, hl = hh // 2, hh % 2
                    head = 4 * g + hh
                    nc.tensor.matmul(
                        out=yd[64 * hl : 64 * (hl + 1), pr, r:qn],
                        lhsT=vnat[0:kn, j, head, :, :, :],
                        rhs=esl[0:kn, hl, r:qn],
                        start=(j == jfirst),
                        stop=(j == jlast),
                        tile_position=(0, 64 * hl),
                        skip_group_check=True,
                    )
                cur = nxt
            # normalization: broadcast denominator lanes, approx-recip, mult
            rs = rr.tile([128, 2, 512], f32, name="rs", tag="rs", bufs=2)
            nc.vector.stream_shuffle(
                out=rs[:, :, 0:qn], in_=yd[:, :, 0:qn], mask=shuf
            )
            rq = rr.tile([128, 2, 512], f32, name="rq", tag="rq", bufs=2)
            nc.vector.reciprocal_approx_fast(out=rq[:, :, 0:qn], in_=rs[:, :, 0:qn])
            nc.vector.tensor_tensor(
                out=ytx_s[:, 2 * g : 2 * g + 2, q0 : q0 + qn],
                in0=yd[:, :, 0:qn],
                in1=rq[:, :, 0:qn],
                op=ALU.mult,
            )

        def emit_outproj(qi):
            # column-split so evac/DMA pipeline; DMAs split over both rings
            q0, qn = Q_TILES[qi]
            half = (qn + 1) // 2
            for m in range(2):
                ops = ps.tile([128, 512], f32, name="ops", tag="s4", bufs=3)
                for ci, (c0, cn) in enumerate([(0, half), (half, qn - half)]):
                    for sl in range(4):
                        nc.tensor.matmul(
                            out=ops[:, c0 : c0 + cn],
                            lhsT=wpx_s[:, sl, m * 128 : (m + 1) * 128],
                            rhs=ytx_s[:, sl, q0 + c0 : q0 + c0 + cn],
                            start=(sl == 0),
                            stop=(sl == 3),
                        )
                    ost = ot.tile([128, 256], bf16, name="ost", tag="ost")
                    nc.scalar.add(
                        ost[:, 0:cn], ops[:, c0 : c0 + cn], bia_s[:, 4 + m : 5 + m]
                    )
                    eng = nc.sync if (m + ci) % 2 == 0 else nc.scalar
                    eng.dma_start(
                        out=out_d[:, m, q0 + c0 : q0 + c0 + cn], in_=ost[:, 0:cn]
                    )

        emit_proj(0, list(range(0, 4)))
        emit_proj(1, list(range(4, 8)))
        emit_proj(2, list(range(8, 12)))
        pj_ctx.__exit__(None, None, None)
        ps = stack.enter_context(tc.tile_pool(name="ps", bufs=1, space="PSUM"))
        emit_attn(0, 0)
        emit_attn(0, 1)
        emit_attn(1, 0)
        emit_outproj(0)
        emit_attn(1, 1)
        emit_attn(2, 0)
        emit_outproj(1)
        emit_attn(2, 1)
        emit_outproj(2)

    nc.compile()
    return nc


def _get_nc():
    if "nc" not in _CACHE:
        _CACHE["nc"] = _build()
    return _CACHE["nc"]


def _make_in_maps(inputs):
    f = np.float32
    x = np.asarray(inputs["x"], f)
    Wq = np.asarray(inputs["Wq"], f)
    Wk = np.asarray(inputs["Wk"], f)
    Wv = np.asarray(inputs["Wv"], f)
    Wp = np.asarray(inputs["Wp"], f)
    bq = np.asarray(inputs["bq"], f)
    bk = np.asarray(inputs["bk"], f)
    bv = np.asarray(inputs["bv"], f)
    bp = np.asarray(inputs["bp"], f)

    import ml_dtypes

    bf = ml_dtypes.bfloat16
    tri = np.triu(np.ones((128, 128), f))  # keep where k-row <= q-col

    # Wp^T rows permuted to the scattered y^T-slab layout (v/ones interleave)
    wpx = np.zeros((128, 4, C), f)
    for i in range(4):
        g, pr = divmod(i, 2)
        for p in range(128):
            hl, ppp = divmod(p, 64)
            head = 4 * g + 2 * pr + hl
            half, inner = divmod(ppp, 32)
            if inner < 16:
                d = half * 16 + inner
                wpx[p, i, :] = Wp[:, head * 32 + d]

    wqt = np.ascontiguousarray((Wq.T * ALPHA).reshape(2, 128, C).transpose(1, 0, 2))
    wkt = np.ascontiguousarray(Wk.T.reshape(2, 128, C).transpose(1, 0, 2))
    wvt = np.ascontiguousarray(Wv.T.reshape(2, 128, C).transpose(1, 0, 2))

    bia = np.zeros((128, 6), f)
    bia[:, 0] = bq[0:128] * ALPHA
    bia[:, 1] = bq[128:256] * ALPHA
    bia[:, 2] = bk[0:128]
    bia[:, 3] = bk[128:256]
    bia[:, 4] = bp[0:128]
    bia[:, 5] = bp[128:256]

    bvm = np.zeros((128, 512), f)
    bvm[:, 0:256] = np.tile(bv, (128, 1))
    bvm[:, 256:384] = tri
    bvm[:, 384:512] = tri

    common = {
        "wq": wqt.astype(bf),
        "wk": wkt.astype(bf),
        "wv": wvt.astype(bf),
        "wpx": np.ascontiguousarray(wpx).astype(bf),
        "bia": np.ascontiguousarray(bia),
        "bvm": np.ascontiguousarray(bvm).astype(bf),
    }
    maps = []
    for b in range(N_CORES):
        xtb = x[b].T.reshape(2, 128, T).transpose(1, 0, 2)  # [128, 2, T]
        m = dict(common)
        for i, (n0, nn) in enumerate(Q_TILES):
            m[f"xt{i}"] = np.ascontiguousarray(xtb[:, :, n0 : n0 + nn]).astype(bf)
        maps.append(m)
    return maps


def run(inputs, trace=False):
    from concourse.bass_utils import run_bass_kernel_spmd

    nc = _get_nc()
    in_maps = _make_in_maps(inputs)
    res = run_bass_kernel_spmd(nc, in_maps, list(range(N_CORES)), trace=trace)
    outs = []
    for i in range(N_CORES):
        ot = np.asarray(res.results[i]["outT"], dtype=np.float32)  # [128, 2, T]
        outs.append(ot.transpose(2, 1, 0).reshape(T, C))
    return np.stack(outs, axis=0), res


def kernel(**inputs) -> np.ndarray:
    out, _ = run(inputs, trace=False)
    return out


# revision 14
# speedup vs baseline: 1.1693x; 1.0379x over previous
"""Causal self-attention (B=8, T=1500, C=256, H=8, D=32) on 8 trn2 NeuronCores.

Sharding: data-parallel over batch B — core b computes batch element b
end-to-end (no collectives). The host only re-lays-out inputs (transposes /
replication); every FLOP of the module runs on device.

v2 changes vs baseline (148us):
  - exp split across ScalarE (native spline exp) and VectorE (Schraudolph
    bit-trick: psum + B -> int16 -> bitcast bf16), removing the single-engine
    exp wall (81.8us serialized on ScalarE).  The 1/sqrt(D)*log2e*128 factor
    is folded into Wq on the host so the DVE op is a single tensor_scalar.
  - S matmuls as 16 32x32 array tiles (4 heads x 4 k-substrips) for full
    array concurrency instead of 4 row-tiled 32x128 matmuls.
  - output projection computed transposed (out^T[c,t]) so the bias add is
    per-partition on ScalarE and the result DMAs as bf16 (host re-transposes).
  - yd has no memset: first PV matmul per region uses start=True.
  - all input DMAs as large contiguous transfers on both HWDGE rings
    (sync+scalar); gpsimd only does memsets; dense PE warmup for HAM ramp.
"""

import numpy as np

B, T, C = 8, 1500, 256
H, D = 8, 32
SCALE = 1.0 / float(np.sqrt(D))
LOG2E = 1.4426950408889634
ALPHA = SCALE * LOG2E * 128.0          # folded into Wq/bq host-side
EXP_SCALE = float(np.log(2.0) / 128.0)  # ScalarE exp scale on alpha-scores
SCHRAUD_B = 16251.0                     # 127*128 + c, c=-5 calibrated
FRAC_DVE = 0.6                          # fraction of s4b columns on DVE
S_TILE16 = True
N_CORES = 8

Q_TILES = [(0, 512), (512, 512), (1024, 476)]
K_TILES = [(j * 128, min(128, T - j * 128)) for j in range(12)]
T_TILES = K_TILES

_CACHE = {}


def _build():
    import concourse.bass as bass
    import concourse.mybir as mybir
    import concourse.tile as tile
    from concourse import bacc

    f32 = mybir.dt.float32
    bf16 = mybir.dt.bfloat16
    i16 = mybir.dt.int16
    AF = mybir.ActivationFunctionType
    ALU = mybir.AluOpType

    nc = bacc.Bacc()

    xt_d = [
        nc.dram_tensor(f"xt{i}", [128, 2, nn], bf16, kind="ExternalInput")
        for i, (n0, nn) in enumerate(Q_TILES)
    ]
    wq_d = nc.dram_tensor("wq", [128, 2, C], bf16, kind="ExternalInput")
    wk_d = nc.dram_tensor("wk", [128, 2, C], bf16, kind="ExternalInput")
    wv_d = nc.dram_tensor("wv", [128, 2, C], bf16, kind="ExternalInput")
    wpx_d = nc.dram_tensor("wpx", [128, 4, C], bf16, kind="ExternalInput")
    bia_d = nc.dram_tensor("bia", [128, 6], f32, kind="ExternalInput")
    bvm_d = nc.dram_tensor("bvm", [128, 512], bf16, kind="ExternalInput")
    out_d = nc.dram_tensor("outT", [128, 2, T], bf16, kind="ExternalOutput")

    from contextlib import ExitStack

    with tile.TileContext(nc) as tc, ExitStack() as stack:
        pp = stack.enter_context(tc.tile_pool(name="persist", bufs=1))
        xt = pp.tile([128, 2, T], bf16, name="xt")
        wq_s = pp.tile([128, 2, C], bf16, name="wq_s")
        wk_s = pp.tile([128, 2, C], bf16, name="wk_s")
        wv_s = pp.tile([128, 2, C], bf16, name="wv_s")
        wpx_s = pp.tile([128, 4, C], bf16, name="wpx_s")
        bia_s = pp.tile([128, 6], f32, name="bia_s")
        bv_s = pp.tile([128, C], bf16, name="bv_s")
        msk_s = pp.tile([128, 2, 128], bf16, name="msk_s")
        qt0 = pp.tile([128, T], bf16, name="qt0")
        qt1 = pp.tile([128, T], bf16, name="qt1")
        kt0 = pp.tile([128, T], bf16, name="kt0")
        kt1 = pp.tile([128, T], bf16, name="kt1")
        qt, kt = [qt0, qt1], [kt0, kt1]
        # v + ones columns: per k-block, per head, 64 cols =
        # [v_d0-15 | 1s x16 | v_d16-31 | 1s x16] so PV also yields denominators
        vnat = pp.tile([128, 12, 8, 2, 2, 16], bf16, name="vnat")
        ytx_s = pp.tile([128, 4, T], bf16, name="ytx_s")
        warm = pp.tile([128, 640], bf16, name="warm")
        warm2 = pp.tile([128, 8], bf16, name="warm2")

        # ---------------- memsets on gpsimd (frees DVE) ----------------
        nc.gpsimd.memset(warm2[:, :], 0.125)
        nc.gpsimd.memset(warm[:, :], 0.125)
        for tt in range(12):
            nc.gpsimd.memset(vnat[:, tt, :, :, :, :], 1.0)

        # ---------------- input DMAs: big transfers, both HWDGE rings -----
        nc.sync.dma_start(out=wq_s, in_=wq_d[:, :, :])
        nc.scalar.dma_start(out=wk_s, in_=wk_d[:, :, :])
        nc.scalar.dma_start(out=bia_s, in_=bia_d[:, :])
        nc.sync.dma_start(out=xt[:, 0:1, 0:512], in_=xt_d[0][:, 0:1, :])
        nc.scalar.dma_start(out=xt[:, 1:2, 0:512], in_=xt_d[0][:, 1:2, :])
        nc.sync.dma_start(out=wv_s, in_=wv_d[:, :, :])
        nc.sync.dma_start(
            out=msk_s[:, :, :],
            in_=bvm_d[:, 256:512].rearrange("p (a b) -> p a b", a=2),
        )
        nc.sync.dma_start(out=bv_s, in_=bvm_d[:, 0:256])
        for i, (n0, nn) in list(enumerate(Q_TILES))[1:]:
            nc.sync.dma_start(out=xt[:, 0:1, n0 : n0 + nn], in_=xt_d[i][:, 0:1, :])
            nc.scalar.dma_start(out=xt[:, 1:2, n0 : n0 + nn], in_=xt_d[i][:, 1:2, :])
        nc.scalar.dma_start(out=wpx_s, in_=wpx_d[:, :, :])

        # warm the ACT exp table before the real exps need it
        nc.scalar.activation(warm2[:, 4:8], warm2[:, 0:4], AF.Exp)

        # ---------------- PSUM pools ----------------
        es = stack.enter_context(tc.tile_pool(name="es", bufs=1))
        rr = stack.enter_context(tc.tile_pool(name="rr", bufs=2))
        ot = stack.enter_context(tc.tile_pool(name="ot", bufs=3))
        pj_ctx = tc.tile_pool(name="pj", bufs=1, space="PSUM")
        pjp = [pj_ctx.__enter__()]

        # dense warmup matmuls: ramp HAM to 2.4GHz during the DMA window
        wmm = pjp[0].tile([128, 512], f32, name="wmm", tag="pj", bufs=8)
        for _ in range(8):
            nc.tensor.matmul(
                out=wmm[:, 0:512],
                lhsT=warm[:, 0:128],
                rhs=warm[:, 128:640],
                start=True,
                stop=True,
            )

        def _ptile():
            return pjp[0].tile([128, 512], f32, name="pt", tag="pj", bufs=8)

        def emit_proj(n, vts):
            n0, nn = Q_TILES[n]
            for m in range(2):
                qp = _ptile()
                for kk in range(2):
                    nc.tensor.matmul(
                        out=qp[:, 0:nn],
                        lhsT=wq_s[:, kk, m * 128 : (m + 1) * 128],
                        rhs=xt[:, kk, n0 : n0 + nn],
                        start=(kk == 0),
                        stop=(kk == 1),
                    )
                nc.scalar.add(qt[m][:, n0 : n0 + nn], qp[:, 0:nn], bia_s[:, m : m + 1])
                kp = _ptile()
                for kk in range(2):
                    nc.tensor.matmul(
                        out=kp[:, 0:nn],
                        lhsT=wk_s[:, kk, m * 128 : (m + 1) * 128],
                        rhs=xt[:, kk, n0 : n0 + nn],
                        start=(kk == 0),
                        stop=(kk == 1),
                    )
                nc.scalar.add(
                    kt[m][:, n0 : n0 + nn], kp[:, 0:nn], bia_s[:, 2 + m : 3 + m]
                )
                for tt in vts[m::2]:
                    t0, tl = T_TILES[tt]
                    vp = _ptile()
                    for kk in range(2):
                        nc.tensor.matmul(
                            out=vp[0:tl, 0:C],
                            lhsT=xt[:, kk, t0 : t0 + tl],
                            rhs=wv_s[:, kk, :],
                            start=(kk == 0),
                            stop=(kk == 1),
                        )
                    nc.vector.tensor_tensor(
                        out=vnat[0:tl, tt, :, :, 0, :],
                        in0=vp[0:tl, 0:C].rearrange(
                            "p (h half d) -> p h half d", h=8, half=2
                        ),
                        in1=bv_s[0:tl, :].rearrange(
                            "p (h half d) -> p h half d", h=8, half=2
                        ),
                        op=ALU.add,
                    )

        shuf = [16 + (i % 16) for i in range(32)]

        def emit_attn(qi, g):
            q0, qn = Q_TILES[qi]
            yd = [
                ps.tile([128, 512], f32, name=f"yd{pr}", tag="yd", bufs=2)
                for pr in range(2)
            ]

            js = [j for j, (k0, kn) in enumerate(K_TILES) if k0 <= q0 + qn - 1]
            jlast = js[-1]
            jfirst = js[0]

            def emit_S(j):
                k0, kn = K_TILES[j]
                r = max(0, k0 - q0)
                s4a = ps.tile([128, 2, 512], f32, name="s4a", tag="s4", bufs=3)
                s4b = ps.tile([128, 2, 512], f32, name="s4b", tag="s4", bufs=3)
                if S_TILE16:
                    # ss-outer so consecutive LDWEIGHTS hit different row
                    # groups (overlap with in-flight matmuls); 16 concurrent
                    # 32x32 array tiles
                    for ss in range(4):
                        ms = min(32, kn - 32 * ss)
                        if ms <= 0:
                            break
                        for hh in range(4):
                            dst = s4a if hh < 2 else s4b
                            nc.tensor.matmul(
                                out=dst[32 * ss : 32 * ss + ms, hh % 2, r:qn],
                                lhsT=kt[g][
                                    32 * hh : 32 * (hh + 1),
                                    k0 + 32 * ss : k0 + 32 * ss + ms,
                                ],
                                rhs=qt[g][32 * hh : 32 * (hh + 1), q0 + r : q0 + qn],
                                start=True,
                                stop=True,
                                tile_position=(32 * hh, 32 * ss),
                            )
                else:
                    for hh in range(4):
                        dst = s4a if hh < 2 else s4b
                        nc.tensor.matmul(
                            out=dst[0:kn, hh % 2, r:qn],
                            lhsT=kt[g][32 * hh : 32 * (hh + 1), k0 : k0 + kn],
                            rhs=qt[g][32 * hh : 32 * (hh + 1), q0 + r : q0 + qn],
                            start=True,
                            stop=True,
                            tile_position=(32 * hh, 0),
                        )
                return s4a, s4b

            cur = emit_S(js[0])
            for idx, j in enumerate(js):
                k0, kn = K_TILES[j]
                r = max(0, k0 - q0)
                diag = k0 >= q0
                w = min(kn, qn - r) if diag else 0
                nxt = emit_S(js[idx + 1]) if idx + 1 < len(js) else None
                s4a, s4b = cur
                esl_a = es.tile([128, 2, 512], bf16, name="esl_a", tag="esl_a", bufs=3)
                esl_b = es.tile([128, 2, 512], bf16, name="esl_b", tag="esl_b", bufs=3)
                # column split of s4b between ScalarE (exact) and DVE (approx)
                mid = r + int(np.ceil((qn - r) * (1.0 - FRAC_DVE)))
                if diag:
                    mid = max(mid, r + w)
                mid = min(mid, qn)
                nc.scalar.activation(
                    out=esl_a[0:kn, :, r:qn], in_=s4a[0:kn, :, r:qn],
                    func=AF.Exp, scale=EXP_SCALE,
                )
                if mid > r:
                    nc.scalar.activation(
                        out=esl_b[0:kn, :, r:mid], in_=s4b[0:kn, :, r:mid],
                        func=AF.Exp, scale=EXP_SCALE,
                    )
                if mid < qn:
                    nc.vector.tensor_scalar(
                        out=esl_b[0:kn, :, mid:qn].bitcast(i16),
                        in0=s4b[0:kn, :, mid:qn],
                        scalar1=SCHRAUD_B,
                        scalar2=0.0,
                        op0=ALU.add,
                        op1=ALU.max,
                    )
                if diag:
                    for esl in (esl_a, esl_b):
                        nc.vector.tensor_tensor(
                            out=esl[0:kn, :, r : r + w],
                            in0=esl[0:kn, :, r : r + w],
                            in1=msk_s[0:kn, :, 0:w],
                            op=ALU.mult,
                        )
                for hh in range(4):
                    esl = esl_a if hh < 2 else esl_b
                    pr, hl = hh // 2, hh % 2
                    head = 4 * g + hh
                    nc.tensor.matmul(
                        out=yd[pr][64 * hl : 64 * (hl + 1), r:qn],
                        lhsT=vnat[0:kn, j, head, :, :, :],
                        rhs=esl[0:kn, hl, r:qn],
                        start=(j == jfirst),
                        stop=(j == jlast),
                        tile_position=(0, 64 * hl),
                        skip_group_check=True,
                    )
                cur = nxt
            # normalization: broadcast denominator lanes, approx-recip, mult
            for pr in range(2):
                rs = rr.tile([128, 512], f32, name="rs", tag="rs", bufs=2)
                nc.vector.stream_shuffle(
                    out=rs[:, 0:qn], in_=yd[pr][:, 0:qn], mask=shuf
                )
                rq = rr.tile([128, 512], f32, name="rq", tag="rq", bufs=2)
                nc.vector.reciprocal_approx_fast(out=rq[:, 0:qn], in_=rs[:, 0:qn])
                nc.vector.tensor_tensor(
                    out=ytx_s[:, 2 * g + pr, q0 : q0 + qn],
                    in0=yd[pr][:, 0:qn],
                    in1=rq[:, 0:qn],
                    op=ALU.mult,
                )

        def emit_outproj(qi):
            # column-split so evac/DMA pipeline; DMAs split over both rings
            q0, qn = Q_TILES[qi]
            half = (qn + 1) // 2
            for m in range(2):
                ops = ps.tile([128, 512], f32, name="ops", tag="s4", bufs=3)
                for ci, (c0, cn) in enumerate([(0, half), (half, qn - half)]):
                    for sl in range(4):
                        nc.tensor.matmul(
                            out=ops[:, c0 : c0 + cn],
                            lhsT=wpx_s[:, sl, m * 128 : (m + 1) * 128],
                            rhs=ytx_s[:, sl, q0 + c0 : q0 + c0 + cn],
                            start=(sl == 0),
                            stop=(sl == 3),
                        )
                    ost = ot.tile([128, 256], bf16, name="ost", tag="ost")
                    nc.scalar.add(
                        ost[:, 0:cn], ops[:, c0 : c0 + cn], bia_s[:, 4 + m : 5 + m]
                    )
                    eng = nc.sync if (m + ci) % 2 == 0 else nc.scalar
                    eng.dma_start(
                        out=out_d[:, m, q0 + c0 : q0 + c0 + cn], in_=ost[:, 0:cn]
                    )

        emit_proj(0, list(range(0, 4)))
        emit_proj(1, list(range(4, 8)))
        emit_proj(2, list(range(8, 12)))
        pj_ctx.__exit__(None, None, None)
        ps = stack.enter_context(tc.tile_pool(name="ps", bufs=1, space="PSUM"))
        emit_attn(0, 0)
        emit_attn(0, 1)
        emit_attn(1, 0)
        emit_outproj(0)
        emit_attn(1, 1)
        emit_attn(2, 0)
        emit_outproj(1)
        emit_attn(2, 1)
        emit_outproj(2)

    nc.compile()
    return nc


def _get_nc():
    if "nc" not in _CACHE:
        _CACHE["nc"] = _build()
    return _CACHE["nc"]


def _make_in_maps(inputs):
    f = np.float32
    x = np.asarray(inputs["x"], f)
    Wq = np.asarray(inputs["Wq"], f)
    Wk = np.asarray(inputs["Wk"], f)
    Wv = np.asarray(inputs["Wv"], f)
    Wp = np.asarray(inputs["Wp"], f)
    bq = np.asarray(inputs["bq"], f)
    bk = np.asarray(inputs["bk"], f)
    bv = np.asarray(inputs["bv"], f)
    bp = np.asarray(inputs["bp"], f)

    import ml_dtypes

    bf = ml_dtypes.bfloat16
    tri = np.triu(np.ones((128, 128), f))  # keep where k-row <= q-col

    # Wp^T rows permuted to the scattered y^T-slab layout (v/ones interleave)
    wpx = np.zeros((128, 4, C), f)
    for i in range(4):
        g, pr = divmod(i, 2)
        for p in range(128):
            hl, ppp = divmod(p, 64)
            head = 4 * g + 2 * pr + hl
            half, inner = divmod(ppp, 32)
            if inner < 16:
                d = half * 16 + inner
                wpx[p, i, :] = Wp[:, head * 32 + d]

    wqt = np.ascontiguousarray((Wq.T * ALPHA).reshape(2, 128, C).transpose(1, 0, 2))
    wkt = np.ascontiguousarray(Wk.T.reshape(2, 128, C).transpose(1, 0, 2))
    wvt = np.ascontiguousarray(Wv.T.reshape(2, 128, C).transpose(1, 0, 2))

    bia = np.zeros((128, 6), f)
    bia[:, 0] = bq[0:128] * ALPHA
    bia[:, 1] = bq[128:256] * ALPHA
    bia[:, 2] = bk[0:128]
    bia[:, 3] = bk[128:256]
    bia[:, 4] = bp[0:128]
    bia[:, 5] = bp[128:256]

    bvm = np.zeros((128, 512), f)
    bvm[:, 0:256] = np.tile(bv, (128, 1))
    bvm[:, 256:384] = tri
    bvm[:, 384:512] = tri

    common = {
        "wq": wqt.astype(bf),
        "wk": wkt.astype(bf),
        "wv": wvt.astype(bf),
        "wpx": np.ascontiguousarray(wpx).astype(bf),
        "bia": np.ascontiguousarray(bia),
        "bvm": np.ascontiguousarray(bvm).astype(bf),
    }
    maps = []
    for b in range(N_CORES):
        xtb = x[b].T.reshape(2, 128, T).transpose(1, 0, 2)  # [128, 2, T]
        m = dict(common)
        for i, (n0, nn) in enumerate(Q_TILES):
            m[f"xt{i}"] = np.ascontiguousarray(xtb[:, :, n0 : n0 + nn]).astype(bf)
        maps.append(m)
    return maps


def run(inputs, trace=False):
    from concourse.bass_utils import run_bass_kernel_spmd

    nc = _get_nc()
    in_maps = _make_in_maps(inputs)
    res = run_bass_kernel_spmd(nc, in_maps, list(range(N_CORES)), trace=trace)
    outs = []
    for i in range(N_CORES):
        ot = np.asarray(res.results[i]["outT"], dtype=np.float32)  # [128, 2, T]
        outs.append(ot.transpose(2, 1, 0).reshape(T, C))
    return np.stack(outs, axis=0), res


def kernel(**inputs) -> np.ndarray:
    out, _ = run(inputs, trace=False)
    return out


# revision 16
# speedup vs baseline: 1.2602x; 1.0777x over previous
"""Causal self-attention (B=8, T=1500, C=256, H=8, D=32) on 8 trn2 NeuronCores.

Sharding: data-parallel over batch B — core b computes batch element b
end-to-end (no collectives). The host only re-lays-out inputs (transposes /
replication); every FLOP of the module runs on device.

v2 changes vs baseline (148us):
  - exp split across ScalarE (native spline exp) and VectorE (Schraudolph
    bit-trick: psum + B -> int16 -> bitcast bf16), removing the single-engine
    exp wall (81.8us serialized on ScalarE).  The 1/sqrt(D)*log2e*128 factor
    is folded into Wq on the host so the DVE op is a single tensor_scalar.
  - S matmuls as 16 32x32 array tiles (4 heads x 4 k-substrips) for full
    array concurrency instead of 4 row-tiled 32x128 matmuls.
  - output projection computed transposed (out^T[c,t]) so the bias add is
    per-partition on ScalarE and the result DMAs as bf16 (host re-transposes).
  - yd has no memset: first PV matmul per region uses start=True.
  - all input DMAs as large contiguous transfers on both HWDGE rings
    (sync+scalar); gpsimd only does memsets; dense PE warmup for HAM ramp.
"""

import numpy as np

B, T, C = 8, 1500, 256
H, D = 8, 32
SCALE = 1.0 / float(np.sqrt(D))
LOG2E = 1.4426950408889634
ALPHA = SCALE * LOG2E * 128.0          # folded into Wq/bq host-side
EXP_SCALE = float(np.log(2.0) / 128.0)  # ScalarE exp scale on alpha-scores
SCHRAUD_B = 16251.0                     # 127*128 + c, c=-5 calibrated
FRAC_DVE = 0.6                          # fraction of s4b columns on DVE
S_TILE16 = True
N_CORES = 8

Q_TILES = [(0, 512), (512, 512), (1024, 476)]
K_TILES = [(j * 128, min(128, T - j * 128)) for j in range(12)]
T_TILES = K_TILES

_CACHE = {}


def _build():
    import concourse.bass as bass
    import concourse.mybir as mybir
    import concourse.tile as tile
    from concourse import bacc

    f32 = mybir.dt.float32
    bf16 = mybir.dt.bfloat16
    i16 = mybir.dt.int16
    AF = mybir.ActivationFunctionType
    ALU = mybir.AluOpType

    nc = bacc.Bacc()

    xt_d = [
        nc.dram_tensor(f"xt{i}", [128, 2, nn], bf16, kind="ExternalInput")
        for i, (n0, nn) in enumerate(Q_TILES)
    ]
    wq_d = nc.dram_tensor("wq", [128, 2, C], bf16, kind="ExternalInput")
    wk_d = nc.dram_tensor("wk", [128, 2, C], bf16, kind="ExternalInput")
    wv_d = nc.dram_tensor("wv", [128, 2, C], bf16, kind="ExternalInput")
    wpx_d = nc.dram_tensor("wpx", [128, 4, C], bf16, kind="ExternalInput")
    bia_d = nc.dram_tensor("bia", [128, 6], f32, kind="ExternalInput")
    bvm_d = nc.dram_tensor("bvm", [128, 512], bf16, kind="ExternalInput")
    out_d = nc.dram_tensor("outT", [128, 2, T], bf16, kind="ExternalOutput")

    from contextlib import ExitStack

    with tile.TileContext(nc) as tc, ExitStack() as stack:
        pp = stack.enter_context(tc.tile_pool(name="persist", bufs=1))
        xt = pp.tile([128, 2, T], bf16, name="xt")
        wq_s = pp.tile([128, 2, C], bf16, name="wq_s")
        wk_s = pp.tile([128, 2, C], bf16, name="wk_s")
        wv_s = pp.tile([128, 2, C], bf16, name="wv_s")
        wpx_s = pp.tile([128, 4, C], bf16, name="wpx_s")
        bia_s = pp.tile([128, 6], f32, name="bia_s")
        bv_s = pp.tile([128, C], bf16, name="bv_s")
        msk_s = pp.tile([128, 2, 128], bf16, name="msk_s")
        qt0 = pp.tile([128, T], bf16, name="qt0")
        qt1 = pp.tile([128, T], bf16, name="qt1")
        kt0 = pp.tile([128, T], bf16, name="kt0")
        kt1 = pp.tile([128, T], bf16, name="kt1")
        qt, kt = [qt0, qt1], [kt0, kt1]
        # v + ones columns: per k-block, per head, 64 cols =
        # [v_d0-15 | 1s x16 | v_d16-31 | 1s x16] so PV also yields denominators
        vnat = pp.tile([128, 12, 8, 2, 2, 16], bf16, name="vnat")
        ytx_s = pp.tile([128, 4, T], bf16, name="ytx_s")
        warm = pp.tile([128, 640], bf16, name="warm")
        warm2 = pp.tile([128, 8], bf16, name="warm2")

        # ---------------- memsets on gpsimd (frees DVE) ----------------
        nc.gpsimd.memset(warm2[:, :], 0.125)
        nc.gpsimd.memset(warm[:, :], 0.125)
        for tt in range(12):
            nc.gpsimd.memset(vnat[:, tt, :, :, :, :], 1.0)

        # ---------------- input DMAs: big transfers, both HWDGE rings -----
        nc.sync.dma_start(out=wq_s, in_=wq_d[:, :, :])
        nc.scalar.dma_start(out=wk_s, in_=wk_d[:, :, :])
        nc.scalar.dma_start(out=bia_s, in_=bia_d[:, :])
        nc.sync.dma_start(out=xt[:, 0:1, 0:512], in_=xt_d[0][:, 0:1, :])
        nc.scalar.dma_start(out=xt[:, 1:2, 0:512], in_=xt_d[0][:, 1:2, :])
        nc.sync.dma_start(out=wv_s, in_=wv_d[:, :, :])
        nc.sync.dma_start(
            out=msk_s[:, :, :],
            in_=bvm_d[:, 256:512].rearrange("p (a b) -> p a b", a=2),
        )
        nc.sync.dma_start(out=bv_s, in_=bvm_d[:, 0:256])
        for i, (n0, nn) in list(enumerate(Q_TILES))[1:]:
            nc.sync.dma_start(out=xt[:, 0:1, n0 : n0 + nn], in_=xt_d[i][:, 0:1, :])
            nc.scalar.dma_start(out=xt[:, 1:2, n0 : n0 + nn], in_=xt_d[i][:, 1:2, :])
        nc.scalar.dma_start(out=wpx_s, in_=wpx_d[:, :, :])

        # warm the ACT exp table before the real exps need it
        nc.scalar.activation(warm2[:, 4:8], warm2[:, 0:4], AF.Exp)

        # ---------------- PSUM pools ----------------
        es = stack.enter_context(tc.tile_pool(name="es", bufs=1))
        rr = stack.enter_context(tc.tile_pool(name="rr", bufs=2))
        ot = stack.enter_context(tc.tile_pool(name="ot", bufs=3))
        pj_ctx = tc.tile_pool(name="pj", bufs=1, space="PSUM")
        pjp = [pj_ctx.__enter__()]

        # dense warmup matmuls: ramp HAM to 2.4GHz during the DMA window
        wmm = pjp[0].tile([128, 512], f32, name="wmm", tag="pj", bufs=8)
        for _ in range(12):
            nc.tensor.matmul(
                out=wmm[:, 0:512],
                lhsT=warm[:, 0:128],
                rhs=warm[:, 128:640],
                start=True,
                stop=True,
            )

        def _ptile():
            return pjp[0].tile([128, 512], f32, name="pt", tag="pj", bufs=8)

        def emit_proj(n, vts):
            n0, nn = Q_TILES[n]
            for m in range(2):
                qp = _ptile()
                for kk in range(2):
                    nc.tensor.matmul(
                        out=qp[:, 0:nn],
                        lhsT=wq_s[:, kk, m * 128 : (m + 1) * 128],
                        rhs=xt[:, kk, n0 : n0 + nn],
                        start=(kk == 0),
                        stop=(kk == 1),
                    )
                nc.scalar.add(qt[m][:, n0 : n0 + nn], qp[:, 0:nn], bia_s[:, m : m + 1])
                kp = _ptile()
                for kk in range(2):
                    nc.tensor.matmul(
                        out=kp[:, 0:nn],
                        lhsT=wk_s[:, kk, m * 128 : (m + 1) * 128],
                        rhs=xt[:, kk, n0 : n0 + nn],
                        start=(kk == 0),
                        stop=(kk == 1),
                    )
                nc.scalar.add(
                    kt[m][:, n0 : n0 + nn], kp[:, 0:nn], bia_s[:, 2 + m : 3 + m]
                )
                for tt in vts[m::2]:
                    t0, tl = T_TILES[tt]
                    vp = _ptile()
                    for kk in range(2):
                        nc.tensor.matmul(
                            out=vp[0:tl, 0:C],
                            lhsT=xt[:, kk, t0 : t0 + tl],
                            rhs=wv_s[:, kk, :],
                            start=(kk == 0),
                            stop=(kk == 1),
                        )
                    nc.vector.tensor_tensor(
                        out=vnat[0:tl, tt, :, :, 0, :],
                        in0=vp[0:tl, 0:C].rearrange(
                            "p (h half d) -> p h half d", h=8, half=2
                        ),
                        in1=bv_s[0:tl, :].rearrange(
                            "p (h half d) -> p h half d", h=8, half=2
                        ),
                        op=ALU.add,
                    )

        shuf = [16 + (i % 16) for i in range(32)]

        def emit_attn(qi, g):
            q0, qn = Q_TILES[qi]
            yd = [
                ps.tile([128, 512], f32, name=f"yd{pr}", tag="yd", bufs=2)
                for pr in range(2)
            ]

            js = [j for j, (k0, kn) in enumerate(K_TILES) if k0 <= q0 + qn - 1]
            jlast = js[-1]
            jfirst = js[0]

            def emit_S(j):
                k0, kn = K_TILES[j]
                r = max(0, k0 - q0)
                s4a = ps.tile([128, 2, 512], f32, name="s4a", tag="s4", bufs=3)
                s4b = ps.tile([128, 2, 512], f32, name="s4b", tag="s4", bufs=3)
                if S_TILE16:
                    # ss-outer so consecutive LDWEIGHTS hit different row
                    # groups (overlap with in-flight matmuls); 8 concurrent
                    # 32x64 array tiles (4 heads x 2 k-substrips)
                    for ss in range(2):
                        ms = min(64, kn - 64 * ss)
                        if ms <= 0:
                            break
                        for hh in range(4):
                            dst = s4a if hh < 2 else s4b
                            nc.tensor.matmul(
                                out=dst[64 * ss : 64 * ss + ms, hh % 2, r:qn],
                                lhsT=kt[g][
                                    32 * hh : 32 * (hh + 1),
                                    k0 + 64 * ss : k0 + 64 * ss + ms,
                                ],
                                rhs=qt[g][32 * hh : 32 * (hh + 1), q0 + r : q0 + qn],
                                start=True,
                                stop=True,
                                tile_position=(32 * hh, 64 * ss),
                            )
                else:
                    for hh in range(4):
                        dst = s4a if hh < 2 else s4b
                        nc.tensor.matmul(
                            out=dst[0:kn, hh % 2, r:qn],
                            lhsT=kt[g][32 * hh : 32 * (hh + 1), k0 : k0 + kn],
                            rhs=qt[g][32 * hh : 32 * (hh + 1), q0 + r : q0 + qn],
                            start=True,
                            stop=True,
                            tile_position=(32 * hh, 0),
                        )
                return s4a, s4b

            cur = emit_S(js[0])
            for idx, j in enumerate(js):
                k0, kn = K_TILES[j]
                r = max(0, k0 - q0)
                diag = k0 >= q0
                w = min(kn, qn - r) if diag else 0
                nxt = emit_S(js[idx + 1]) if idx + 1 < len(js) else None
                s4a, s4b = cur
                esl_a = es.tile([128, 2, 512], bf16, name="esl_a", tag="esl_a", bufs=3)
                esl_b = es.tile([128, 2, 512], bf16, name="esl_b", tag="esl_b", bufs=3)
                # column split of s4b between ScalarE (exact) and DVE (approx)
                mid = r + int(np.ceil((qn - r) * (1.0 - FRAC_DVE)))
                if diag:
                    mid = max(mid, r + w)
                mid = min(mid, qn)
                nc.scalar.activation(
                    out=esl_a[0:kn, :, r:qn], in_=s4a[0:kn, :, r:qn],
                    func=AF.Exp, scale=EXP_SCALE,
                )
                if mid > r:
                    nc.scalar.activation(
                        out=esl_b[0:kn, :, r:mid], in_=s4b[0:kn, :, r:mid],
                        func=AF.Exp, scale=EXP_SCALE,
                    )
                if mid < qn:
                    nc.vector.tensor_scalar(
                        out=esl_b[0:kn, :, mid:qn].bitcast(i16),
                        in0=s4b[0:kn, :, mid:qn],
                        scalar1=SCHRAUD_B,
                        scalar2=0.0,
                        op0=ALU.add,
                        op1=ALU.max,
                    )
                if diag:
                    for esl in (esl_a, esl_b):
                        nc.vector.tensor_tensor(
                            out=esl[0:kn, :, r : r + w],
                            in0=esl[0:kn, :, r : r + w],
                            in1=msk_s[0:kn, :, 0:w],
                            op=ALU.mult,
                        )
                for hh in range(4):
                    esl = esl_a if hh < 2 else esl_b
                    pr, hl = hh // 2, hh % 2
                    head = 4 * g + hh
                    nc.tensor.matmul(
                        out=yd[pr][64 * hl : 64 * (hl + 1), r:qn],
                        lhsT=vnat[0:kn, j, head, :, :, :],
                        rhs=esl[0:kn, hl, r:qn],
                        start=(j == jfirst),
                        stop=(j == jlast),
                        tile_position=(0, 64 * hl),
                        skip_group_check=True,
                    )
                cur = nxt
            # normalization: broadcast denominator lanes, approx-recip, mult
            for pr in range(2):
                rs = rr.tile([128, 512], f32, name="rs", tag="rs", bufs=2)
                nc.vector.stream_shuffle(
                    out=rs[:, 0:qn], in_=yd[pr][:, 0:qn], mask=shuf
                )
                rq = rr.tile([128, 512], f32, name="rq", tag="rq", bufs=2)
                nc.vector.reciprocal_approx_fast(out=rq[:, 0:qn], in_=rs[:, 0:qn])
                nc.vector.tensor_tensor(
                    out=ytx_s[:, 2 * g + pr, q0 : q0 + qn],
                    in0=yd[pr][:, 0:qn],
                    in1=rq[:, 0:qn],
                    op=ALU.mult,
                )

        def emit_outproj(qi):
            # column-split so evac/DMA pipeline; DMAs split over both rings
            q0, qn = Q_TILES[qi]
            half = (qn + 1) // 2
            for m in range(2):
                ops = ps.tile([128, 512], f32, name="ops", tag="s4", bufs=3)
                for ci, (c0, cn) in enumerate([(0, half), (half, qn - half)]):
                    for sl in range(4):
                        nc.tensor.matmul(
                            out=ops[:, c0 : c0 + cn],
                            lhsT=wpx_s[:, sl, m * 128 : (m + 1) * 128],
                            rhs=ytx_s[:, sl, q0 + c0 : q0 + c0 + cn],
                            start=(sl == 0),
                            stop=(sl == 3),
                        )
                    ost = ot.tile([128, 256], bf16, name="ost", tag="ost")
                    nc.scalar.add(
                        ost[:, 0:cn], ops[:, c0 : c0 + cn], bia_s[:, 4 + m : 5 + m]
                    )
                    eng = nc.sync if (m + ci) % 2 == 0 else nc.scalar
                    eng.dma_start(
                        out=out_d[:, m, q0 + c0 : q0 + c0 + cn], in_=ost[:, 0:cn]
                    )

        emit_proj(0, list(range(0, 4)))
        emit_proj(1, list(range(4, 8)))
        emit_proj(2, list(range(8, 12)))
        pj_ctx.__exit__(None, None, None)
        ps = stack.enter_context(tc.tile_pool(name="ps", bufs=1, space="PSUM"))
        emit_attn(0, 0)
        emit_attn(0, 1)
        emit_attn(1, 0)
        emit_outproj(0)
        emit_attn(1, 1)
        emit_attn(2, 0)
        emit_outproj(1)
        emit_attn(2, 1)
        emit_outproj(2)

    nc.compile()
    return nc


def _get_nc():
    if "nc" not in _CACHE:
        _CACHE["nc"] = _build()
    return _CACHE["nc"]


def _make_in_maps(inputs):
    f = np.float32
    x = np.asarray(inputs["x"], f)
    Wq = np.asarray(inputs["Wq"], f)
    Wk = np.asarray(inputs["Wk"], f)
    Wv = np.asarray(inputs["Wv"], f)
    Wp = np.asarray(inputs["Wp"], f)
    bq = np.asarray(inputs["bq"], f)
    bk = np.asarray(inputs["bk"], f)
    bv = np.asarray(inputs["bv"], f)
    bp = np.asarray(inputs["bp"], f)

    import ml_dtypes

    bf = ml_dtypes.bfloat16
    tri = np.triu(np.ones((128, 128), f))  # keep where k-row <= q-col

    # Wp^T rows permuted to the scattered y^T-slab layout (v/ones interleave)
    wpx = np.zeros((128, 4, C), f)
    for i in range(4):
        g, pr = divmod(i, 2)
        for p in range(128):
            hl, ppp = divmod(p, 64)
            head = 4 * g + 2 * pr + hl
            half, inner = divmod(ppp, 32)
            if inner < 16:
                d = half * 16 + inner
                wpx[p, i, :] = Wp[:, head * 32 + d]

    wqt = np.ascontiguousarray((Wq.T * ALPHA).reshape(2, 128, C).transpose(1, 0, 2))
    wkt = np.ascontiguousarray(Wk.T.reshape(2, 128, C).transpose(1, 0, 2))
    wvt = np.ascontiguousarray(Wv.T.reshape(2, 128, C).transpose(1, 0, 2))

    bia = np.zeros((128, 6), f)
    bia[:, 0] = bq[0:128] * ALPHA
    bia[:, 1] = bq[128:256] * ALPHA
    bia[:, 2] = bk[0:128]
    bia[:, 3] = bk[128:256]
    bia[:, 4] = bp[0:128]
    bia[:, 5] = bp[128:256]

    bvm = np.zeros((128, 512), f)
    bvm[:, 0:256] = np.tile(bv, (128, 1))
    bvm[:, 256:384] = tri
    bvm[:, 384:512] = tri

    common = {
        "wq": wqt.astype(bf),
        "wk": wkt.astype(bf),
        "wv": wvt.astype(bf),
        "wpx": np.ascontiguousarray(wpx).astype(bf),
        "bia": np.ascontiguousarray(bia),
        "bvm": np.ascontiguousarray(bvm).astype(bf),
    }
    maps = []
    for b in range(N_CORES):
        xtb = x[b].T.reshape(2, 128, T).transpose(1, 0, 2)  # [128, 2, T]
        m = dict(common)
        for i, (n0, nn) in enumerate(Q_TILES):
            m[f"xt{i}"] = np.ascontiguousarray(xtb[:, :, n0 : n0 + nn]).astype(bf)
        maps.append(m)
    return maps


def run(inputs, trace=False):
    from concourse.bass_utils import run_bass_kernel_spmd

    nc = _get_nc()
    in_maps = _make_in_maps(inputs)
    res = run_bass_kernel_spmd(nc, in_maps, list(range(N_CORES)), trace=trace)
    outs = []
    for i in range(N_CORES):
        ot = np.asarray(res.results[i]["outT"], dtype=np.float32)  # [128, 2, T]
        outs.append(ot.transpose(2, 1, 0).reshape(T, C))
    return np.stack(outs, axis=0), res


def kernel(**inputs) -> np.ndarray:
    out, _ = run(inputs, trace=False)
    return out
